# revision 17
# baseline (speedup 1.0000x reference)
"""Trainium2 Bass kernel for nn_DynamicShortConvolution.

Reference computation (per token t, channel d):
    h    = silu(x @ w1)                       # [T, H]
    flat = h @ w2 + b2                        # [T, D*W]
    k    = flat.reshape(T, D, W)
    out[t, d] = silu(sum_w k[t, d, w] * x[t - (W-1) + w, d])

Sharding: 8 cores, each one (batch, half-of-T) shard of 2048 tokens plus a
3-token left halo.  All per-core tensors are laid out TRANSPOSED ([D, T],
channels on SBUF partitions) so the conv's token shift is a free-dim offset.

v3: the kernel is elementwise-bound (PSUM evacuation), so the unit of work
is one (d-tile, chunk-PAIR) = 1024 tokens.  Iterating dt-outer within a
chunk pair makes the four tap windows of x contiguous across the two
chunks, so every evacuation op is a single wide [128,1024] instruction
with a uniform per-partition b2 bias:
  - tensor: per unit 16 matmuls fill four [128,1024] psum tiles (one per
    tap, halves = the two chunks), psum ring of 3 tiles + 2 mm1 banks,
  - ACT pulls taps 0,2 as [128,1024] Identity(+bias) ops to bf16,
  - GPSIMD multiplies those by their x windows (tensor_tensor mult;
    GPSIMD cannot read PSUM and has no stt opcode),
  - DVE does wide f32 stt (bias+mul) for taps 1,3 from PSUM, the
    [128,2048] pair-add and the [128,1024] final add (bf16 2x mode),
  - ACT silu [128,1024] -> one output DMA per unit.
Emission is skewed (adds one unit late, silu two units late) so the
cross-engine chain gp -> adds -> silu -> next pull never serializes.
mm1 for the next pair's chunks is spread 2 contraction-tiles per unit
(units 2-9 and 8-15, matching x chunk DMA arrival), and input DMA is
ordered w1, x0, x1, b2, w2[dt0-7], x2, w2[dt8-15], x3 to minimize the
lead-in (first matmul needs only w1 + x0).
"""

import numpy as np

# Problem constants (hardcoded per harness contract).
B, T, D, H, W = 4, 4096, 2048, 256, 4
HALO = W - 1
N_CORES = 8
TOK = (B * T) // N_CORES  # tokens per core = 2048
TCH = 512                 # token chunk (psum bank = 512 fp32)


def _build_nc(tok, d, h, xstride, out_f32=False, sim_safe=False):
    """Build the single-core Bass/Tile program."""
    import concourse.bass as bass
    import concourse.bacc as bacc
    import concourse.mybir as mybir
    import concourse.tile as tile

    f32 = mybir.dt.float32
    bf16 = mybir.dt.bfloat16
    AF = mybir.ActivationFunctionType
    ALU = mybir.AluOpType
    # CoreSim has no Silu; Sigmoid exercises the identical dataflow
    AF_ACT = AF.Sigmoid if sim_safe else AF.Silu

    n_dt = d // 128        # d tiles = 16
    n_hc = h // 128        # h tiles = 2
    n_tc = tok // TCH      # token chunks = 4
    n_cp = n_tc // 2       # chunk pairs = 2
    UW = 2 * TCH           # unit width = 1024 tokens

    nc = bacc.Bacc()

    # DRAM I/O (host-prepared layouts)
    xT = nc.declare_dram_parameter("xT", [n_dt, 128, xstride], bf16, isOutput=False)
    w1 = nc.declare_dram_parameter("w1", [n_dt, 128, h], bf16, isOutput=False)
    # w2r[hc, hl, dt, w, dl] = w2[hc*128+hl, ((dt*128+dl))*W + w]
    w2r = nc.declare_dram_parameter("w2r", [n_hc, 128, n_dt, W, 128], bf16,
                                    isOutput=False)
    # b2r[p, dt*W + w] = b2[(dt*128+p)*W + w]
    b2r = nc.declare_dram_parameter("b2r", [128, n_dt * W], f32, isOutput=False)
    out_dt = f32 if out_f32 else bf16
    # outT[p, (cp*n_dt + dt)*UW + j] = out token cp*UW+j, channel dt*128+p
    outT = nc.declare_dram_parameter("outT", [128, n_tc * n_dt * TCH], out_dt,
                                     isOutput=True)

    with tile.TileContext(nc) as tc:
        with (
            tc.tile_pool(name="resident", bufs=1) as rpool,
            tc.tile_pool(name="work", bufs=3) as wpool,
            tc.tile_pool(name="psum2", bufs=3, space="PSUM") as ppool,
            tc.tile_pool(name="psumH", bufs=2, space="PSUM") as hpool,
        ):
            # ---- resident tiles ----
            xT_sb = rpool.tile([128, n_dt * xstride], bf16, tag="xT")
            w1_sb = rpool.tile([128, n_dt * h], bf16, tag="w1")
            w2_sb = rpool.tile([128, n_hc, n_dt, W, 128], bf16, tag="w2")
            b2_sb = rpool.tile([128, n_dt * W], f32, tag="b2")
            # hT chunk-major: [hc0 512 | hc1 512] per chunk
            hT_sb = rpool.tile([128, n_tc * 2 * TCH], bf16, tag="hT")

            # ---- input DMA, lead-in ordered ----
            def dma_x_chunk(c):
                a = 0 if c == 0 else c * TCH + HALO
                bnd = c * TCH + TCH + HALO
                for dt in range(n_dt):
                    nc.sync.dma_start(
                        xT_sb[:, dt * xstride + a: dt * xstride + bnd],
                        xT[dt][:, a:bnd])

            def dma_w2(dts):
                for dt in dts:
                    for hc in range(n_hc):
                        nc.sync.dma_start(w2_sb[:, hc, dt], w2r[hc][:, dt])

            for dt in range(n_dt):
                nc.sync.dma_start(w1_sb[:, dt * h:(dt + 1) * h], w1[dt])
            dma_x_chunk(0)
            dma_x_chunk(1)
            nc.sync.dma_start(b2_sb[:], b2r[:])
            dma_w2(range(0, n_dt // 2))
            dma_x_chunk(2)
            dma_w2(range(n_dt // 2, n_dt))
            dma_x_chunk(3)

            def x_slice(dt, col, n):
                return xT_sb[:, dt * xstride + col: dt * xstride + col + n]

            def b2s(dt, w):
                return b2_sb[:, dt * W + w: dt * W + w + 1]

            def hslice(c, hc):
                return hT_sb[:, c * 1024 + hc * TCH: c * 1024 + (hc + 1) * TCH]

            def mm1_mms(c, hps, q):
                # contraction tile q of mm1 for chunk c
                for hc in range(n_hc):
                    nc.tensor.matmul(
                        hps[hc][:],
                        w1_sb[:, q * h + hc * 128: q * h + hc * 128 + 128],
                        x_slice(q, HALO + c * TCH, TCH),
                        start=(q == 0), stop=(q == n_dt - 1),
                    )

            def mm1_silu(c, hps):
                for hc in range(n_hc):
                    nc.scalar.activation(hslice(c, hc), hps[hc][:], AF_ACT)

            def hps_alloc():
                tiles = []
                for hc in range(n_hc):
                    hpt = hpool.tile([128, TCH], f32, tag="hps", name=f"hps{hc}")
                    tiles.append(hpt)
                return tiles

            # ---- chunks 0,1 mm1 up front ----
            for c in range(2):
                hps = hps_alloc()
                for q in range(n_dt):
                    mm1_mms(c, hps, q)
                mm1_silu(c, hps)

            sw_q = []    # (mbuf, unit): pair-add + final add, 1-unit skew
            silu_q = []  # (finbuf, unit): awaiting silu, 2-unit skew

            def emit_sw():
                while sw_q:
                    mb, u = sw_q.pop(0)
                    sb = wpool.tile([128, 2048], bf16, tag="s")
                    nc.vector.tensor_tensor(
                        sb[:], mb[:, :2048], mb[:, 2048:], op=ALU.add)
                    fb = wpool.tile([128, 1024], bf16, tag="fin")
                    nc.vector.tensor_tensor(
                        fb[:], sb[:, :1024], sb[:, 1024:], op=ALU.add)
                    silu_q.append((fb, u))

            def emit_silu(now):
                while silu_q and (now is None or silu_q[0][1] <= now - 2):
                    fb, u = silu_q.pop(0)
                    ot = wpool.tile([128, 1024], out_dt, tag="ot")
                    nc.scalar.activation(ot[:], fb[:], AF_ACT)
                    nc.sync.dma_start(outT[:, u * UW: u * UW + UW], ot[:])

            for cp in range(n_cp):
                j0 = cp * UW
                # mm1 spread plan for the next pair's chunks
                cA = 2 * cp + 2 if cp + 1 < n_cp else None
                hpsA = hps_alloc() if cA is not None else None
                hpsB = hps_alloc() if cA is not None else None
                for dt in range(n_dt):
                    u = cp * n_dt + dt
                    # tensor: four [128,1024] tap tiles; halves = chunks.
                    # Taps 0,2 first so the ACT pulls (which feed the gp
                    # multiplies) start as early as possible.
                    pt = {}
                    for w in (0, 2, 1, 3):
                        pt[w] = ppool.tile([128, 1024], f32, tag="ps",
                                           name=f"p{w}")
                        for ci in range(2):
                            c = 2 * cp + ci
                            for hc in range(n_hc):
                                nc.tensor.matmul(
                                    pt[w][:, ci * TCH:(ci + 1) * TCH],
                                    w2_sb[:, hc, dt, w],
                                    hslice(c, hc),
                                    start=(hc == 0), stop=(hc == n_hc - 1),
                                )
                    # tensor: spread mm1 for the next pair (x-DMA paced;
                    # cB strictly after silu(cA) frees the 2-deep hps ring)
                    if cA is not None:
                        if 2 <= dt < 10:
                            q = 2 * (dt - 2)
                            mm1_mms(cA, hpsA, q)
                            mm1_mms(cA, hpsA, q + 1)
                        elif dt >= 10:
                            for q in range(3 * (dt - 10), min(3 * (dt - 9), n_dt)):
                                mm1_mms(cA + 1, hpsB, q)
                    # ACT: pull taps 0,2 wide with the b2 bias fused
                    qbuf = wpool.tile([128, 2048], bf16, tag="q")
                    nc.scalar.activation(qbuf[:, :1024], pt[0][:],
                                         AF.Identity, bias=b2s(dt, 0))
                    nc.scalar.activation(qbuf[:, 1024:], pt[2][:],
                                         AF.Identity, bias=b2s(dt, 2))
                    # GPSIMD: multiply pulled taps by x windows
                    mbuf = wpool.tile([128, 4096], bf16, tag="m")
                    nc.gpsimd.tensor_tensor(
                        mbuf[:, 0:1024], qbuf[:, :1024],
                        x_slice(dt, j0 + 0, UW), op=ALU.mult)
                    nc.gpsimd.tensor_tensor(
                        mbuf[:, 2048:3072], qbuf[:, 1024:],
                        x_slice(dt, j0 + 2, UW), op=ALU.mult)
                    # DVE: adds for the previous unit first (their inputs
                    # are ready; this unit's p1/p3 land late on tensor)
                    emit_sw()
                    emit_silu(u)
                    # DVE: wide f32 stt for taps 1,3
                    nc.vector.scalar_tensor_tensor(
                        mbuf[:, 1024:2048], pt[1][:], b2s(dt, 1),
                        x_slice(dt, j0 + 1, UW), op0=ALU.add, op1=ALU.mult)
                    nc.vector.scalar_tensor_tensor(
                        mbuf[:, 3072:4096], pt[3][:], b2s(dt, 3),
                        x_slice(dt, j0 + 3, UW), op0=ALU.add, op1=ALU.mult)
                    sw_q.append((mbuf, u))
                    # ACT: hT silu for the next pair's chunks
                    if cA is not None and dt == 9:
                        mm1_silu(cA, hpsA)
                    if cA is not None and dt == n_dt - 1:
                        mm1_silu(cA + 1, hpsB)
            emit_sw()
            emit_silu(None)
    nc.compile()
    return nc


def _prep_shards(x, w1, w2, b2, tok, d, h, halo, xstride):
    """Host-side shard prep. Returns list of per-core in_maps."""
    import ml_dtypes
    bf16 = ml_dtypes.bfloat16

    n_dt = d // 128
    n_hc = h // 128
    b, t, _ = x.shape
    shards_per_batch = (b * t // tok) // b
    w1_r = np.ascontiguousarray(
        w1.reshape(n_dt, 128, h)).astype(bf16)
    # w2 [h, d*W] -> [n_hc, 128, n_dt, W, 128]
    w2_r = np.ascontiguousarray(
        w2.reshape(n_hc, 128, n_dt, 128, W)
        .transpose(0, 1, 2, 4, 3)).astype(bf16)
    b2_r = np.ascontiguousarray(
        b2.reshape(n_dt, 128, W).transpose(1, 0, 2)
        .reshape(128, n_dt * W)).astype(np.float32)

    in_maps = []
    for core in range(N_CORES):
        bi, half = divmod(core, shards_per_batch)
        t0 = half * tok
        xh = np.zeros((tok + halo, d), np.float32)
        lo = max(t0 - halo, 0)
        xh[halo - (t0 - lo):] = x[bi, lo: t0 + tok]
        xTc = np.zeros((n_dt, 128, xstride), bf16)
        xTc[:, :, : tok + halo] = (
            xh.T.astype(bf16).reshape(n_dt, 128, tok + halo))
        in_maps.append({
            "xT": xTc, "w1": w1_r, "w2r": w2_r, "b2r": b2_r})
    return in_maps


_NC_CACHE = {}


def kernel(x, w1, w2, b2, trace=False):
    from concourse.bass_utils import run_bass_kernel_spmd

    tok, d, h = TOK, D, H
    xstride = tok + HALO + 1  # even -> keeps bf16 4B alignment per dtile
    key = (tok, d, h)
    if key not in _NC_CACHE:
        _NC_CACHE[key] = _build_nc(tok, d, h, xstride=xstride)
    nc = _NC_CACHE[key]

    in_maps = _prep_shards(
        np.asarray(x, np.float32), np.asarray(w1, np.float32),
        np.asarray(w2, np.float32), np.asarray(b2, np.float32),
        tok, d, h, HALO, xstride)

    res = run_bass_kernel_spmd(nc, in_maps, core_ids=list(range(N_CORES)),
                               trace=trace)
    kernel.last_result = res

    n_dt = d // 128
    n_cp = (tok // TCH) // 2
    shards_per_batch = (B * T // tok) // B
    out = np.empty((B, T, D), np.float32)
    for core in range(N_CORES):
        bi, half = divmod(core, shards_per_batch)
        oT = res.results[core]["outT"]  # [128, n_cp*n_dt*1024]
        # [128p, cp, dt, j] -> [cp, j, dt, p] -> [tok, d]
        o = oT.reshape(128, n_cp, n_dt, 1024).transpose(1, 3, 2, 0)
        out[bi, half * tok:(half + 1) * tok] = (
            o.reshape(tok, d).astype(np.float32))
    return out
